# revision 1
# baseline (speedup 1.0000x reference)
"""Trainium2 Bass kernel for nn_ContrastiveLoss (SimCLR + spatial contrastive loss).

Strategy (8-core data parallel):
  - Host: L2-normalize z1/z2/embeddings (fp32), build transposed bf16 operand
    tables, gather anchor rows, compute fp32 positive-pair dots.
  - Device (per core): bf16 matmuls of its 1024 simclr rows and 512 spatial
    rows against the full 8192-column tables; fused exp(x/T) + row-sum on the
    ACT engine straight out of PSUM; a tiny PE Gram matmul per row-tile whose
    diagonal reproduces bit-exactly the self-similarity terms, which are
    exp'd identically and returned as per-row corrections.
  - Host: sum_exp = S_raw - corr (exact diagonal/anchor-column removal),
    log, subtract positives, mean-reduce -> [2] losses.

Self-contained: hardcodes shapes from the problem spec.
"""
import sys

for _p in ("/opt/trn_rl_repo", "/root/.axon_site/_ro/trn_rl_repo"):
    if _p not in sys.path:
        sys.path.insert(0, _p)

import numpy as np
import ml_dtypes

import concourse.tile as tile
from concourse import bacc, mybir
from concourse.bass_utils import run_bass_kernel_spmd

TEMPERATURE = 0.07
B = 4096     # simclr batch
D = 256      # projection dim
N = 8192     # num cells (spatial table rows, also 2B simclr table rows)
P = 4096     # num spatial pairs
NCORES = 8
SR = B // NCORES          # 512 simclr pair-rows per core (=> 1024 sim rows)
PR = P // NCORES          # 512 spatial rows per core
RT_SIMCLR = (2 * SR) // 128   # 8 row-tiles
RT_SPATIAL = PR // 128        # 4 row-tiles
RT_TOTAL = RT_SIMCLR + RT_SPATIAL  # 12
NCHUNK = N // 512         # 16 column chunks of 512
NGROUP = 4                # psum groups of 2048 columns
F32 = mybir.dt.float32
BF16 = mybir.dt.bfloat16

_CACHE = {}


def _build_nc():
    nc = bacc.Bacc("TRN2", target_bir_lowering=False)

    zT = nc.dram_tensor("zT", [128, 2, N], BF16, kind="ExternalInput")
    eT = nc.dram_tensor("eT", [128, 2, N], BF16, kind="ExternalInput")
    zTl = nc.dram_tensor("zTl", [128, 2, 2 * SR], BF16, kind="ExternalInput")
    aTl = nc.dram_tensor("aTl", [128, 2, PR], BF16, kind="ExternalInput")
    ident = nc.dram_tensor("ident", [128, 128], F32, kind="ExternalInput")

    sraw_o = nc.dram_tensor("sraw", [128, RT_TOTAL], F32, kind="ExternalOutput")
    corr_o = nc.dram_tensor("corr", [128, RT_TOTAL], F32, kind="ExternalOutput")

    inv_t = float(1.0 / np.float32(TEMPERATURE))

    with tile.TileContext(nc) as tc:
        with (
            tc.tile_pool(name="tabs", bufs=1) as tabs,
            tc.tile_pool(name="psum", bufs=2, space="PSUM") as psum,
            tc.tile_pool(name="scr", bufs=2) as scrp,
            tc.tile_pool(name="small", bufs=1) as small,
            tc.tile_pool(name="tmp", bufs=4) as tmpp,
        ):
            # Small operand tables first so PE can start (grams) immediately;
            # big tables split per 2048-column group so the first main matmul
            # group only waits on its own 1MB chunk.
            zTl_t = tabs.tile([128, 2, 2 * SR], BF16)
            aTl_t = tabs.tile([128, 2, PR], BF16)
            ident_t = small.tile([128, 128], F32)
            # Group 0 of zT lives in four 512-column sub-tiles so the very
            # first matmuls/exps only wait for 0.25MB of DMA, not 3.2MB.
            zT_c = [tabs.tile([128, 2, 512], BF16, name=f"zTc{j}")
                    for j in range(4)]
            zT_g = [None] + [tabs.tile([128, 2, 2048], BF16, name=f"zTg{g}")
                             for g in range(1, NGROUP)]
            eT_g = [tabs.tile([128, 2, 2048], BF16, name=f"eTg{g}")
                    for g in range(NGROUP)]
            # Load order = consumption order: lhsT slices, then the first rhs
            # chunks (critical path of the first matmul group), then the rest.
            nc.sync.dma_start(zTl_t[:], zTl[:])
            nc.sync.dma_start(aTl_t[:], aTl[:])
            for j in range(4):
                nc.sync.dma_start(zT_c[j][:], zT[:, :, j * 512:(j + 1) * 512])
            nc.sync.dma_start(ident_t[:], ident[:])
            for g in range(1, NGROUP):
                nc.sync.dma_start(zT_g[g][:], zT[:, :, g * 2048:(g + 1) * 2048])
            for g in range(NGROUP):
                nc.sync.dma_start(eT_g[g][:], eT[:, :, g * 2048:(g + 1) * 2048])

            sraw_t = small.tile([128, RT_TOTAL], F32)
            corr_t = small.tile([128, RT_TOTAL], F32)

            def lhsT_pair(rt):
                lh, li = (zTl_t, rt) if rt < RT_SIMCLR else (aTl_t, rt - RT_SIMCLR)
                return (lh[:, 0, li * 128:(li + 1) * 128],
                        lh[:, 1, li * 128:(li + 1) * 128])

            # All Gram diagonals up front: the diagonal of lhsT.T@lhsT is
            # bitwise-identical to the main matmul's self-similarity element
            # for each row; exp'd identically it cancels those terms exactly.
            pgr = psum.tile([128, 2048], F32, tag="big")
            for grt in range(RT_TOTAL):
                l0, l1 = lhsT_pair(grt)
                nc.tensor.matmul(pgr[:, grt * 128:(grt + 1) * 128],
                                 l0, l0, start=True, stop=False)
                nc.tensor.matmul(pgr[:, grt * 128:(grt + 1) * 128],
                                 l1, l1, start=False, stop=True)
            gd_all = tmpp.tile([128, RT_TOTAL, 128], F32, tag="gd")
            for grt in range(RT_TOTAL):
                nc.vector.tensor_tensor(
                    gd_all[:, grt, :],
                    pgr[:, grt * 128:(grt + 1) * 128],
                    ident_t[:], mybir.AluOpType.mult,
                )

            gdv_all = tmpp.tile([128, RT_TOTAL], F32, tag="gdv")
            nc.vector.tensor_reduce(
                gdv_all[:], gd_all[:],
                axis=mybir.AxisListType.X, op=mybir.AluOpType.add,
            )
            nc.scalar.activation(
                corr_t[:], gdv_all[:],
                mybir.ActivationFunctionType.Exp, scale=inv_t,
            )
            nc.sync.dma_start(corr_o[:], corr_t[:])

            # Persistent per-(row-tile, group) partial sums; zeroed once so
            # the final reduce can span unused slots of the fine-grained rt0.
            part_all = small.tile([128, RT_TOTAL, NGROUP + 3], F32)
            nc.vector.memset(part_all[:], 0.0)

            def emit_unit(rt, g):
                """8 matmuls + exp(accum) for one (row-tile, 2048-col group)."""
                lhsT0, lhsT1 = lhsT_pair(rt)
                simclr = rt < RT_SIMCLR
                fine = rt == 0 and g == 0
                pg = psum.tile([128, 2048], F32, tag="big")
                mm_order = ([(kc, cc) for cc in range(4) for kc in range(2)]
                            if fine else
                            [(kc, cc) for kc in range(2) for cc in range(4)])
                for kc, cc in mm_order:
                    lz = lhsT0 if kc == 0 else lhsT1
                    if simclr and g == 0:
                        rhs = zT_c[cc][:, kc, :]
                    else:
                        tab = zT_g[g] if simclr else eT_g[g]
                        rhs = tab[:, kc, cc * 512:(cc + 1) * 512]
                    nc.tensor.matmul(
                        pg[:, cc * 512:(cc + 1) * 512], lz, rhs,
                        start=(kc == 0), stop=(kc == 1),
                    )
                # exp output is dead (only accum_out matters): write it
                # in-place over the PSUM bank.
                if fine:
                    for cc in range(4):
                        nc.scalar.activation(
                            pg[:, cc * 512:(cc + 1) * 512],
                            pg[:, cc * 512:(cc + 1) * 512],
                            mybir.ActivationFunctionType.Exp,
                            scale=inv_t, accum_out=part_all[:, rt, cc:cc + 1],
                        )
                else:
                    # rt0 g1-3 shift past the four fine-grained g0 slots
                    ps = g + 3 if rt == 0 else g
                    nc.scalar.activation(
                        pg[:], pg[:], mybir.ActivationFunctionType.Exp,
                        scale=inv_t, accum_out=part_all[:, rt, ps:ps + 1],
                    )

            # Simclr sweeps group-major: once the first 1MB column group has
            # arrived, all 8 row-tiles can run against it, so ACT never
            # starves during the remaining table DMA. Spatial runs after
            # (eT is fully resident long before it starts).
            for g in range(NGROUP):
                for rt in range(RT_SIMCLR):
                    emit_unit(rt, g)
            for rt in range(RT_SIMCLR, RT_TOTAL):
                for g in range(NGROUP):
                    emit_unit(rt, g)

            nc.vector.tensor_reduce(
                sraw_t[:], part_all[:],
                axis=mybir.AxisListType.X, op=mybir.AluOpType.add,
            )

            nc.sync.dma_start(sraw_o[:], sraw_t[:])

    nc.finalize()
    return nc


def _l2norm(x):
    n = np.maximum(np.linalg.norm(x.astype(np.float32), axis=1, keepdims=True), 1e-12)
    return (x.astype(np.float32) / n).astype(np.float32)


def _pack_T(x):
    """[R, D=256] fp32 -> transposed bf16 operand table [128, 2, R]."""
    xT = np.ascontiguousarray(x.T)                      # [256, R]
    return np.ascontiguousarray(
        xT.reshape(2, 128, xT.shape[1]).transpose(1, 0, 2)
    ).astype(ml_dtypes.bfloat16)


def prepare(z1, z2, embeddings, anchor_idx, neighbor_idx):
    """Host-side prep: returns (in_maps, host_ctx)."""
    z1n = _l2norm(np.asarray(z1))
    z2n = _l2norm(np.asarray(z2))
    en = _l2norm(np.asarray(embeddings))
    ai = np.asarray(anchor_idx).astype(np.int64)
    ni = np.asarray(neighbor_idx).astype(np.int64)

    zcat = np.concatenate([z1n, z2n], axis=0)           # [2B, D]
    zT_p = _pack_T(zcat)                                # [128, 2, 8192] bf16
    eT_p = _pack_T(en)                                  # [128, 2, 8192] bf16
    a_rows = en[ai]                                     # [P, D] fp32
    aT_p = _pack_T(a_rows)                              # [128, 2, 4096] bf16

    # fp32 positive-pair logits (match reference semantics)
    psim = (np.sum(z1n.astype(np.float64) * z2n.astype(np.float64), axis=1)
            / np.float64(np.float32(TEMPERATURE)))      # [B]
    pos = (np.sum(a_rows.astype(np.float64) * en[ni].astype(np.float64), axis=1)
           / np.float64(np.float32(TEMPERATURE)))       # [P]
    eq = (ai == ni).astype(np.float64)                  # [P]

    ident = np.eye(128, dtype=np.float32)
    in_maps = []
    for c in range(NCORES):
        zTl_p = np.ascontiguousarray(np.concatenate(
            [zT_p[:, :, c * SR:(c + 1) * SR],
             zT_p[:, :, B + c * SR:B + (c + 1) * SR]], axis=2))  # [128,2,1024]
        aTl_p = np.ascontiguousarray(aT_p[:, :, c * PR:(c + 1) * PR])  # [128,2,512]
        in_maps.append({
            "zT": zT_p, "eT": eT_p, "zTl": zTl_p, "aTl": aTl_p, "ident": ident,
        })
    return in_maps, (psim, pos, eq)


def finish(results, host_ctx):
    """Host-side epilogue: assemble the two losses from per-core S_raw/corr."""
    psim, pos, eq = host_ctx
    terms1 = np.empty(2 * B, dtype=np.float64)
    terms2 = np.empty(P, dtype=np.float64)
    for c in range(NCORES):
        S = results[c]["sraw"].astype(np.float64).T.reshape(-1)   # [12*128], idx rt*128+p
        C = results[c]["corr"].astype(np.float64).T.reshape(-1)

        s_sim = S[:2 * SR * 1]  # first 8 tiles = 1024 rows
        c_sim = C[:2 * SR]
        sum_exp = s_sim[:2 * SR] - c_sim[:2 * SR]
        p_loc = psim[c * SR:(c + 1) * SR]
        # local rows [0,512) -> z1 part, [512,1024) -> z2 part; same positives
        terms1[c * SR:(c + 1) * SR] = np.log(sum_exp[:SR]) - p_loc
        terms1[B + c * SR:B + (c + 1) * SR] = np.log(sum_exp[SR:2 * SR]) - p_loc

        s_sp = S[2 * SR:2 * SR + PR]
        c_sp = C[2 * SR:2 * SR + PR]
        g = slice(c * PR, (c + 1) * PR)
        total = s_sp - c_sp + eq[g] * np.exp(pos[g])
        terms2[g] = np.log(total) - pos[g]

    l1 = terms1.mean()
    l2 = terms2.mean()
    return np.array([l1, l2], dtype=np.float32)


def get_nc():
    if "nc" not in _CACHE:
        _CACHE["nc"] = _build_nc()
    return _CACHE["nc"]


def kernel(z1, z2, embeddings, anchor_idx, neighbor_idx):
    in_maps, host_ctx = prepare(z1, z2, embeddings, anchor_idx, neighbor_idx)
    nc = get_nc()
    res = run_bass_kernel_spmd(nc, in_maps, list(range(NCORES)))
    return finish(res.results, host_ctx)



# revision 2
# speedup vs baseline: 1.2694x; 1.2694x over previous
"""Trainium2 Bass kernel for nn_ContrastiveLoss (SimCLR + spatial contrastive loss).

Strategy (8-core data parallel, row-oriented):
  - Host: L2-normalize z1/z2/embeddings (fp32), quantize to fp8e4 (e4m3),
    build transposed [128, 2, cols] operand tables, gather anchor rows,
    compute fp64 positive-pair dots.
  - Device (per core): fp8e4 DoubleRow matmuls (full 256-deep contraction in
    one PE pass at 0.5 cyc/row) of its 1024 simclr rows and 512 spatial rows
    against the full 8192-column tables. The exp+rowsum of each [128, W] PSUM
    tile is split across two engines:
      * ACT: fused exp(x/T) with fp32 accum_out (in-place dead write to PSUM)
      * DVE: Schraudolph bit-trick exp — tensor_scalar computes
        round(A*x + B) into int16 (these ARE the bf16 bits of exp(x/T)),
        then a second 4x-mode tensor_scalar over the bf16 bitcast view
        accumulates the row sums.
    A per-row-tile Gram matmul (same fp8 operands, same DoubleRow mode)
    reproduces the self-similarity diagonal bitwise; it is pushed through
    BOTH engines' exp ops so the host can subtract exactly the value that
    entered each rowsum (engine chosen per row by the static unit map).
  - Host: sum_exp = S_raw - corr(engine-matched), log, subtract fp64
    positives, mean-reduce -> [2] losses.

Self-contained: hardcodes shapes from the problem spec.
"""
import sys

for _p in ("/opt/trn_rl_repo", "/root/.axon_site/_ro/trn_rl_repo"):
    if _p not in sys.path:
        sys.path.insert(0, _p)

import numpy as np
import ml_dtypes

import concourse.tile as tile
from concourse import bacc, mybir
from concourse.bass_utils import run_bass_kernel_spmd

TEMPERATURE = 0.07
B = 4096     # simclr batch
D = 256      # projection dim
N = 8192     # num cells (spatial table rows, also 2B simclr table rows)
P = 4096     # num spatial pairs
NCORES = 8
SR = B // NCORES          # 512 simclr pair-rows per core (=> 1024 sim rows)
PR = P // NCORES          # 512 spatial rows per core
RT_SIMCLR = (2 * SR) // 128   # 8 row-tiles
RT = RT_SIMCLR + PR // 128    # 12 row-tiles total

F32 = mybir.dt.float32
BF16 = mybir.dt.bfloat16
I16 = mybir.dt.int16
FP8E4 = mybir.dt.float8e4

INV_T = float(np.float32(1.0) / np.float32(TEMPERATURE))
# Schraudolph constants: bits16 = round(A16*x + B16) are the bf16 bits of
# ~exp(x/T).  badj calibrated so the weighted mean of the sum ratio is 1.
A16 = float(np.float32(128.0 * np.log2(np.e) / np.float64(np.float32(TEMPERATURE))))
B16 = float(np.float32(127.0 * 128.0 - 10.14))

# Column units per row-tile: (start, width); width matches the PSUM tile of
# its rotation slot [1536, 1536, 1024, 1536, 1536, 1024].
UNITS = [(0, 1536), (1536, 1536), (3072, 1024),
         (4096, 1536), (5632, 1536), (7168, 1024)]
UNIT_BOUNDS = [u[0] for u in UNITS] + [N]
NU = len(UNITS)

# Engine patterns balancing ACT (1.05 ns/elem) vs DVE (1.48 ns/elem)
_P1_ACT = (0, 1, 3, 5)   # 5632 elems on ACT, 2560 on DVE
_P2_ACT = (0, 2, 5)      # 3584 on ACT, 4608 on DVE
_P1_RTS = (0, 2, 4, 6, 8, 10, 11)


def eng_of(rt, u):
    acts = _P1_ACT if rt in _P1_RTS else _P2_ACT
    return "A" if u in acts else "D"


def unit_of_col(col):
    return int(np.searchsorted(UNIT_BOUNDS, col, side="right") - 1)


_CACHE = {}


def _build_nc():
    nc = bacc.Bacc("TRN2", target_bir_lowering=False)

    zT = nc.dram_tensor("zT", [128, 2, N], FP8E4, kind="ExternalInput")
    eT = nc.dram_tensor("eT", [128, 2, N], FP8E4, kind="ExternalInput")
    zTl = nc.dram_tensor("zTl", [128, 2, 2 * SR], FP8E4, kind="ExternalInput")
    aTl = nc.dram_tensor("aTl", [128, 2, PR], FP8E4, kind="ExternalInput")
    ident = nc.dram_tensor("ident", [128, 128], F32, kind="ExternalInput")

    sraw_o = nc.dram_tensor("sraw", [128, RT], F32, kind="ExternalOutput")
    corrA_o = nc.dram_tensor("corrA", [128, RT], F32, kind="ExternalOutput")
    corrD_o = nc.dram_tensor("corrD", [128, RT], I16, kind="ExternalOutput")

    NCH = 4          # table DMA chunks of 2048 columns
    DR = mybir.MatmulPerfMode.DoubleRow

    with tile.TileContext(nc) as tc:
        with (
            tc.tile_pool(name="tabs", bufs=1) as tabs,
            tc.tile_pool(name="psum", bufs=1, space="PSUM") as psum,
            tc.tile_pool(name="small", bufs=1) as small,
            tc.tile_pool(name="bits", bufs=2) as bitsp,
        ):
            zTl_t = tabs.tile([128, 2, 2 * SR], FP8E4)
            aTl_t = tabs.tile([128, 2, PR], FP8E4)
            ident_t = small.tile([128, 128], F32)
            zc = [tabs.tile([128, 2, 2048], FP8E4, name=f"zc{j}")
                  for j in range(NCH)]
            ec = [tabs.tile([128, 2, 2048], FP8E4, name=f"ec{j}")
                  for j in range(NCH)]
            # Load order = consumption order.
            nc.sync.dma_start(zTl_t[:], zTl[:])
            nc.sync.dma_start(aTl_t[:], aTl[:])
            nc.sync.dma_start(ident_t[:], ident[:])
            for j in range(NCH):
                nc.sync.dma_start(zc[j][:], zT[:, :, j * 2048:(j + 1) * 2048])
                nc.sync.dma_start(ec[j][:], eT[:, :, j * 2048:(j + 1) * 2048])

            p_tiles = [psum.tile([128, 1536], F32, name="p0"),
                       psum.tile([128, 1536], F32, name="p1"),
                       psum.tile([128, 1024], F32, name="p2")]

            part = small.tile([128, RT, NU], F32)
            gd = small.tile([128, RT, 128], F32)
            gdv = small.tile([128, RT], F32)
            corrA_t = small.tile([128, RT], F32)
            corrD_t = small.tile([128, RT], I16)
            sraw_t = small.tile([128, RT], F32)

            def lhsT(rt):
                if rt < RT_SIMCLR:
                    return zTl_t[:, :, rt * 128:(rt + 1) * 128]
                i = rt - RT_SIMCLR
                return aTl_t[:, :, i * 128:(i + 1) * 128]

            # --- Gram phase: diagonals reproduce the main matmuls'
            # self-similarity elements bitwise (same operands, same mode).
            pg = p_tiles[0]
            for rt in range(RT):
                nc.tensor.matmul(pg[:, rt * 128:(rt + 1) * 128],
                                 lhsT(rt), lhsT(rt), start=True, stop=True,
                                 perf_mode=DR)
            for rt in range(RT):
                nc.vector.tensor_tensor(
                    gd[:, rt, :], pg[:, rt * 128:(rt + 1) * 128],
                    ident_t[:], mybir.AluOpType.mult,
                )
            nc.vector.tensor_reduce(
                gdv[:], gd[:], axis=mybir.AxisListType.X,
                op=mybir.AluOpType.add,
            )
            # Exp the diagonals through BOTH engine paths; host selects.
            nc.scalar.activation(
                corrA_t[:], gdv[:], mybir.ActivationFunctionType.Exp,
                scale=INV_T,
            )
            nc.vector.tensor_scalar(
                corrD_t[:], gdv[:], A16, B16,
                mybir.AluOpType.mult, mybir.AluOpType.add,
            )
            nc.sync.dma_start(corrA_o[:], corrA_t[:])
            nc.sync.dma_start(corrD_o[:], corrD_t[:])

            # --- Main units: bands of 3 units x 12 row-tiles; psum tile
            # rotation keeps PE one tile ahead of the two exp engines.
            for band in range(2):
                for rt in range(RT):
                    for j in range(3):
                        u = band * 3 + j
                        c0, W = UNITS[u]
                        pt = p_tiles[j]
                        tbl = zc if rt < RT_SIMCLR else ec
                        for off in range(0, W, 512):
                            col = c0 + off
                            ch, co = col // 2048, col % 2048
                            nc.tensor.matmul(
                                pt[:, off:off + 512], lhsT(rt),
                                tbl[ch][:, :, co:co + 512],
                                start=True, stop=True, perf_mode=DR,
                            )
                        slot = part[:, rt, u:u + 1]
                        if eng_of(rt, u) == "A":
                            nc.scalar.activation(
                                pt[:, :W], pt[:, :W],
                                mybir.ActivationFunctionType.Exp,
                                scale=INV_T, accum_out=slot,
                            )
                        else:
                            bt = bitsp.tile([128, 1536], I16, tag="bits")
                            nc.vector.tensor_scalar(
                                bt[:, :W], pt[:, :W], A16, B16,
                                mybir.AluOpType.mult, mybir.AluOpType.add,
                            )
                            bb = bt[:, :W].bitcast(BF16)
                            nc.vector.tensor_scalar(
                                bb, bb, 1.0, 0.0,
                                mybir.AluOpType.mult, mybir.AluOpType.add,
                                accum_out=slot,
                            )

            nc.vector.tensor_reduce(
                sraw_t[:], part[:], axis=mybir.AxisListType.X,
                op=mybir.AluOpType.add,
            )
            nc.sync.dma_start(sraw_o[:], sraw_t[:])

    nc.finalize()
    return nc


def _l2norm(x):
    x = np.asarray(x, dtype=np.float32)
    n = np.maximum(np.linalg.norm(x, axis=1, keepdims=True), 1e-12)
    return (x / n).astype(np.float32)


def _pack_T8(xq):
    """[R, D=256] fp8 -> transposed operand table [128, 2, R] (same bytes)."""
    xT = np.ascontiguousarray(xq.T)                      # [256, R]
    return np.ascontiguousarray(
        xT.reshape(2, 128, xT.shape[1]).transpose(1, 0, 2)
    )


def prepare(z1, z2, embeddings, anchor_idx, neighbor_idx):
    """Host-side prep: returns (in_maps, host_ctx)."""
    z1n = _l2norm(z1)
    z2n = _l2norm(z2)
    en = _l2norm(embeddings)
    ai = np.asarray(anchor_idx).astype(np.int64)
    ni = np.asarray(neighbor_idx).astype(np.int64)

    zcat = np.concatenate([z1n, z2n], axis=0)            # [2B, D] fp32
    zq = zcat.astype(ml_dtypes.float8_e4m3)              # quantize once
    eq_ = np.asarray(embeddings, np.float32)
    eq8 = en.astype(ml_dtypes.float8_e4m3)

    zT_p = _pack_T8(zq)                                  # [128, 2, 8192]
    eT_p = _pack_T8(eq8)                                 # [128, 2, 8192]
    a_rows8 = eq8[ai]                                    # [P, D] fp8 (same bytes)
    aT_p = _pack_T8(a_rows8)                             # [128, 2, 4096]

    # fp64 positive-pair logits (match reference semantics: full precision)
    psim = (np.sum(z1n.astype(np.float64) * z2n.astype(np.float64), axis=1)
            / np.float64(np.float32(TEMPERATURE)))       # [B]
    pos = (np.sum(en[ai].astype(np.float64) * en[ni].astype(np.float64), axis=1)
           / np.float64(np.float32(TEMPERATURE)))        # [P]
    eqmask = (ai == ni).astype(np.float64)               # [P]

    ident = np.eye(128, dtype=np.float32)
    in_maps = []
    for c in range(NCORES):
        zTl_p = np.ascontiguousarray(np.concatenate(
            [zT_p[:, :, c * SR:(c + 1) * SR],
             zT_p[:, :, B + c * SR:B + (c + 1) * SR]], axis=2))
        aTl_p = np.ascontiguousarray(aT_p[:, :, c * PR:(c + 1) * PR])
        in_maps.append({
            "zT": zT_p, "eT": eT_p, "zTl": zTl_p, "aTl": aTl_p,
            "ident": ident,
        })
    return in_maps, (psim, pos, eqmask, ai)


def finish(results, host_ctx):
    """Host-side epilogue: assemble the two losses."""
    psim, pos, eqmask, ai = host_ctx
    terms1 = np.empty(2 * B, dtype=np.float64)
    terms2 = np.empty(P, dtype=np.float64)
    lanes = np.arange(128)
    bounds = np.asarray(UNIT_BOUNDS)
    for c in range(NCORES):
        r = results[c]
        sraw = r["sraw"].astype(np.float64)              # [128, 12]
        corrA = r["corrA"].astype(np.float64)
        corrD = (np.asarray(r["corrD"], np.int16)
                 .view(ml_dtypes.bfloat16).astype(np.float64))

        for rt in range(RT_SIMCLR):
            if rt < RT_SIMCLR // 2:
                row0 = c * SR + rt * 128                 # z1 rows
            else:
                row0 = B + c * SR + (rt - RT_SIMCLR // 2) * 128
            u = unit_of_col(row0)                        # whole tile in one unit
            corr = corrA[:, rt] if eng_of(rt, u) == "A" else corrD[:, rt]
            s = sraw[:, rt] - corr
            rows = row0 + lanes
            pair = rows % B                              # psim index
            terms1[rows] = np.log(s) - psim[pair]

        for rt in range(RT_SIMCLR, RT):
            p0_ = c * PR + (rt - RT_SIMCLR) * 128
            pg = p0_ + lanes
            ua = np.searchsorted(bounds, ai[pg], side="right") - 1
            isA = np.array([eng_of(rt, int(u)) == "A" for u in ua])
            corr = np.where(isA, corrA[:, rt], corrD[:, rt])
            tot = sraw[:, rt] - corr + eqmask[pg] * np.exp(pos[pg])
            terms2[pg] = np.log(tot) - pos[pg]

    l1 = terms1.mean()
    l2 = terms2.mean()
    return np.array([l1, l2], dtype=np.float32)


def get_nc():
    if "nc" not in _CACHE:
        _CACHE["nc"] = _build_nc()
    return _CACHE["nc"]


def kernel(z1, z2, embeddings, anchor_idx, neighbor_idx):
    in_maps, host_ctx = prepare(z1, z2, embeddings, anchor_idx, neighbor_idx)
    nc = get_nc()
    res = run_bass_kernel_spmd(nc, in_maps, list(range(NCORES)))
    return finish(res.results, host_ctx)


# revision 15
# speedup vs baseline: 1.4825x; 1.1679x over previous
"""Trainium2 Bass kernel for nn_ContrastiveLoss (SimCLR + spatial contrastive loss).

Strategy (8-core data parallel, row-oriented):
  - Host: L2-normalize z1/z2/embeddings (fp32), quantize to fp8e4 (e4m3),
    build transposed [128, 2, cols] operand tables, gather anchor rows,
    compute fp64 positive-pair dots.
  - Device (per core): fp8e4 DoubleRow matmuls (full 256-deep contraction in
    one PE pass at 0.5 cyc/row) of its 1024 simclr rows and 512 spatial rows
    against the full 8192-column tables. The exp+rowsum of each [128, W] PSUM
    tile is split across two engines:
      * ACT: fused exp(x/T) with fp32 accum_out (in-place dead write to PSUM)
      * DVE: Schraudolph bit-trick exp — tensor_scalar computes
        round(A*x + B) into int16 (these ARE the bf16 bits of exp(x/T)),
        then a second 4x-mode tensor_scalar over the bf16 bitcast view
        accumulates the row sums.
    A per-row-tile Gram matmul (same fp8 operands, same DoubleRow mode)
    reproduces the self-similarity diagonal bitwise; it is pushed through
    BOTH engines' exp ops so the host can subtract exactly the value that
    entered each rowsum (engine chosen per row by the static unit map).
  - Host: sum_exp = S_raw - corr(engine-matched), log, subtract fp64
    positives, mean-reduce -> [2] losses.

Self-contained: hardcodes shapes from the problem spec.
"""
import sys

for _p in ("/opt/trn_rl_repo", "/root/.axon_site/_ro/trn_rl_repo"):
    if _p not in sys.path:
        sys.path.insert(0, _p)

import numpy as np
import ml_dtypes

import concourse.tile as tile
from concourse import bacc, mybir
from concourse.bass_utils import run_bass_kernel_spmd

TEMPERATURE = 0.07
B = 4096     # simclr batch
D = 256      # projection dim
N = 8192     # num cells (spatial table rows, also 2B simclr table rows)
P = 4096     # num spatial pairs
NCORES = 8
SR = B // NCORES          # 512 simclr pair-rows per core (=> 1024 sim rows)
PR = P // NCORES          # 512 spatial rows per core
RT_SIMCLR = (2 * SR) // 128   # 8 row-tiles
RT = RT_SIMCLR + PR // 128    # 12 row-tiles total

F32 = mybir.dt.float32
BF16 = mybir.dt.bfloat16
I16 = mybir.dt.int16
FP8E4 = mybir.dt.float8e4

INV_T = float(np.float32(1.0) / np.float32(TEMPERATURE))
# Schraudolph constants: bits16 = round(A16*x + B16) are the bf16 bits of
# ~exp(x/T).  badj calibrated so the weighted mean of the sum ratio is 1.
A16 = float(np.float32(128.0 * np.log2(np.e) / np.float64(np.float32(TEMPERATURE))))
B16 = float(np.float32(127.0 * 128.0 - 10.14))

# --- static schedule configuration -----------------------------------------
PSUM_SIZES = (1536, 1536, 1024)   # psum rotation tile sizes (512-multiples)
GRAM_AT_END = False
GRAM_TILE = -1         # gram outputs fill the last-used psum tiles first
ALT_PENALTY = 0.0      # ns penalty for repeating the previous stream engine
SPLIT_C0 = False       # DMA the first table chunk in 512-col pieces
POOL_REDUCE = False    # run SBUF-side reduces on GpSimd instead of DVE
BITS_BUFS = 2


def _mk_units():
    cyc = sum(PSUM_SIZES)
    assert N % cyc == 0
    units = []
    col = 0
    for _ in range(N // cyc):
        for s in PSUM_SIZES:
            units.append((col, s))
            col += s
    return units


UNITS = _mk_units()
NU = len(UNITS)
UNIT_BOUNDS = [u[0] for u in UNITS] + [N]
NBANDS = N // sum(PSUM_SIZES)
NTILES = len(PSUM_SIZES)


def _mk_engine_map():
    """Greedy global load balance across ACT / DVE in program order, with an
    optional bias toward alternating engines between consecutive units."""
    def act_cost(w):
        return 0.8333 * w + 330.0

    def dve_cost(w):
        return 1.0417 * w + 0.26 * w + 335.0

    eng = [[None] * NU for _ in range(RT)]
    ta = td = 0.0
    prev = None
    for band in range(NBANDS):
        for rt in range(RT):
            for j in range(NTILES):
                u = band * NTILES + j
                w = UNITS[u][1]
                ca = ta + act_cost(w) + (ALT_PENALTY if prev == "A" else 0.0)
                cd = td + dve_cost(w) + (ALT_PENALTY if prev == "D" else 0.0)
                if ca <= cd:
                    eng[rt][u] = "A"
                    ta += act_cost(w)
                    prev = "A"
                else:
                    eng[rt][u] = "D"
                    td += dve_cost(w)
                    prev = "D"
    return eng


ENG = _mk_engine_map()


def eng_of(rt, u):
    return ENG[rt][u]


def unit_of_col(col):
    return int(np.searchsorted(UNIT_BOUNDS, col, side="right") - 1)


_CACHE = {}


def _build_nc():
    nc = bacc.Bacc("TRN2", target_bir_lowering=False)

    zT = nc.dram_tensor("zT", [128, 2, N], FP8E4, kind="ExternalInput")
    eT = nc.dram_tensor("eT", [128, 2, N], FP8E4, kind="ExternalInput")
    zTl = nc.dram_tensor("zTl", [128, 2, 2 * SR], FP8E4, kind="ExternalInput")
    aTl = nc.dram_tensor("aTl", [128, 2, PR], FP8E4, kind="ExternalInput")
    ident = nc.dram_tensor("ident", [128, 128], F32, kind="ExternalInput")

    sraw_o = nc.dram_tensor("sraw", [128, RT, NU], F32, kind="ExternalOutput")
    corrA_o = nc.dram_tensor("corrA", [128, RT], F32, kind="ExternalOutput")
    corrD_o = nc.dram_tensor("corrD", [128, RT], I16, kind="ExternalOutput")

    NCH = 4          # table DMA chunks of 2048 columns
    DR = mybir.MatmulPerfMode.DoubleRow
    WMAX = max(PSUM_SIZES)

    with tile.TileContext(nc) as tc:
        with (
            tc.tile_pool(name="tabs", bufs=1) as tabs,
            tc.tile_pool(name="psum", bufs=1, space="PSUM") as psum,
            tc.tile_pool(name="small", bufs=1) as small,
            tc.tile_pool(name="bits", bufs=BITS_BUFS) as bitsp,
        ):
            zTl_t = tabs.tile([128, 2, 2 * SR], FP8E4)
            aTl_t = tabs.tile([128, 2, PR], FP8E4)
            ident_t = small.tile([128, 128], F32)
            zc = [tabs.tile([128, 2, 2048], FP8E4, name=f"zc{j}")
                  for j in range(NCH)]
            ec = [tabs.tile([128, 2, 2048], FP8E4, name=f"ec{j}")
                  for j in range(NCH)]
            zc0q = ([tabs.tile([128, 2, 512], FP8E4, name=f"zc0q{i}")
                     for i in range(4)] if SPLIT_C0 else None)
            # Load order = consumption order: lhsT slices first (grams +
            # every unit), then the first simclr chunk the first units read.
            nc.sync.dma_start(zTl_t[:], zTl[:])
            if SPLIT_C0:
                for i in range(4):
                    nc.sync.dma_start(zc0q[i][:],
                                      zT[:, :, i * 512:(i + 1) * 512])
            else:
                nc.sync.dma_start(zc[0][:], zT[:, :, 0:2048])
            nc.sync.dma_start(aTl_t[:], aTl[:])
            nc.sync.dma_start(ident_t[:], ident[:])
            nc.sync.dma_start(ec[0][:], eT[:, :, 0:2048])
            for j in range(1, NCH):
                nc.sync.dma_start(zc[j][:], zT[:, :, j * 2048:(j + 1) * 2048])
                nc.sync.dma_start(ec[j][:], eT[:, :, j * 2048:(j + 1) * 2048])

            def rhs_of(rt, col):
                """[col, col+512) slice of the right table."""
                if rt >= RT_SIMCLR:
                    return ec[col // 2048][:, :, col % 2048:col % 2048 + 512]
                if SPLIT_C0 and col < 2048:
                    return zc0q[col // 512][:]
                return zc[col // 2048][:, :, col % 2048:col % 2048 + 512]

            p_tiles = [psum.tile([128, s], F32, name=f"p{i}")
                       for i, s in enumerate(PSUM_SIZES)]

            part = small.tile([128, RT, NU], F32)
            gd = small.tile([128, RT, 128], F32)
            gdv = small.tile([128, RT], F32)
            corrA_t = small.tile([128, RT], F32)
            corrD_t = small.tile([128, RT], I16)
            sraw_t = small.tile([128, RT], F32)

            def lhsT(rt):
                if rt < RT_SIMCLR:
                    return zTl_t[:, :, rt * 128:(rt + 1) * 128]
                i = rt - RT_SIMCLR
                return aTl_t[:, :, i * 128:(i + 1) * 128]

            def gram_phase():
                # Gram diagonals == main matmuls' self-similarity elements
                # bitwise (same operands, same DoubleRow mode).
                done = 0
                tile_order = (list(range(NTILES))[::-1] if GRAM_TILE < 0
                              else [(GRAM_TILE + k) % NTILES
                                    for k in range(NTILES)])
                for ti in tile_order:
                    pt = p_tiles[ti]
                    cap = PSUM_SIZES[ti] // 128
                    take = min(cap, RT - done)
                    for k in range(take):
                        rt = done + k
                        nc.tensor.matmul(pt[:, k * 128:(k + 1) * 128],
                                         lhsT(rt), lhsT(rt),
                                         start=True, stop=True, perf_mode=DR)
                    for k in range(take):
                        rt = done + k
                        nc.vector.tensor_tensor(
                            gd[:, rt, :], pt[:, k * 128:(k + 1) * 128],
                            ident_t[:], mybir.AluOpType.mult,
                        )
                    done += take
                    if done >= RT:
                        break
                red = nc.gpsimd if POOL_REDUCE else nc.vector
                red.tensor_reduce(
                    gdv[:], gd[:], axis=mybir.AxisListType.X,
                    op=mybir.AluOpType.add,
                )
                # Exp the diagonals through BOTH engine paths; host selects.
                nc.scalar.activation(
                    corrA_t[:], gdv[:], mybir.ActivationFunctionType.Exp,
                    scale=INV_T,
                )
                nc.vector.tensor_scalar(
                    corrD_t[:], gdv[:], A16, B16,
                    mybir.AluOpType.mult, mybir.AluOpType.add,
                )
                nc.sync.dma_start(corrA_o[:], corrA_t[:])
                nc.sync.dma_start(corrD_o[:], corrD_t[:])

            if not GRAM_AT_END:
                gram_phase()

            # --- Main units: rotation over psum tiles keeps PE ahead of the
            # two exp engines.
            for band in range(NBANDS):
                for rt in range(RT):
                    for j in range(NTILES):
                        u = band * NTILES + j
                        c0, W = UNITS[u]
                        pt = p_tiles[j]
                        for off in range(0, W, 512):
                            nc.tensor.matmul(
                                pt[:, off:off + 512], lhsT(rt),
                                rhs_of(rt, c0 + off),
                                start=True, stop=True, perf_mode=DR,
                            )
                        slot = part[:, rt, u:u + 1]
                        if eng_of(rt, u) == "A":
                            nc.scalar.activation(
                                pt[:, :W], pt[:, :W],
                                mybir.ActivationFunctionType.Exp,
                                scale=INV_T, accum_out=slot,
                            )
                        else:
                            bt = bitsp.tile([128, WMAX], I16, tag="bits")
                            nc.vector.tensor_scalar(
                                bt[:, :W], pt[:, :W], A16, B16,
                                mybir.AluOpType.mult, mybir.AluOpType.add,
                            )
                            bb = bt[:, :W].bitcast(BF16)
                            nc.vector.tensor_scalar(
                                bb, bb, 1.0, 0.0,
                                mybir.AluOpType.mult, mybir.AluOpType.add,
                                accum_out=slot,
                            )

            if GRAM_AT_END:
                gram_phase()

            # Ship the per-unit partial sums; host does the final 6-way add.
            nc.sync.dma_start(sraw_o[:], part[:])

    nc.finalize()
    return nc


def _l2norm(x):
    x = np.asarray(x, dtype=np.float32)
    n = np.maximum(np.linalg.norm(x, axis=1, keepdims=True), 1e-12)
    return (x / n).astype(np.float32)


def _pack_T8(xq):
    """[R, D=256] fp8 -> transposed operand table [128, 2, R] (same bytes)."""
    xT = np.ascontiguousarray(xq.T)                      # [256, R]
    return np.ascontiguousarray(
        xT.reshape(2, 128, xT.shape[1]).transpose(1, 0, 2)
    )


def prepare(z1, z2, embeddings, anchor_idx, neighbor_idx):
    """Host-side prep: returns (in_maps, host_ctx)."""
    z1n = _l2norm(z1)
    z2n = _l2norm(z2)
    en = _l2norm(embeddings)
    ai = np.asarray(anchor_idx).astype(np.int64)
    ni = np.asarray(neighbor_idx).astype(np.int64)

    zcat = np.concatenate([z1n, z2n], axis=0)            # [2B, D] fp32
    zq = zcat.astype(ml_dtypes.float8_e4m3)              # quantize once
    eq8 = en.astype(ml_dtypes.float8_e4m3)

    zT_p = _pack_T8(zq)                                  # [128, 2, 8192]
    eT_p = _pack_T8(eq8)                                 # [128, 2, 8192]
    a_rows8 = eq8[ai]                                    # [P, D] fp8 (same bytes)
    aT_p = _pack_T8(a_rows8)                             # [128, 2, 4096]

    # fp64 positive-pair logits (match reference semantics: full precision)
    psim = (np.sum(z1n.astype(np.float64) * z2n.astype(np.float64), axis=1)
            / np.float64(np.float32(TEMPERATURE)))       # [B]
    pos = (np.sum(en[ai].astype(np.float64) * en[ni].astype(np.float64), axis=1)
           / np.float64(np.float32(TEMPERATURE)))        # [P]
    eqmask = (ai == ni).astype(np.float64)               # [P]

    ident = np.eye(128, dtype=np.float32)
    in_maps = []
    for c in range(NCORES):
        zTl_p = np.ascontiguousarray(np.concatenate(
            [zT_p[:, :, c * SR:(c + 1) * SR],
             zT_p[:, :, B + c * SR:B + (c + 1) * SR]], axis=2))
        aTl_p = np.ascontiguousarray(aT_p[:, :, c * PR:(c + 1) * PR])
        in_maps.append({
            "zT": zT_p, "eT": eT_p, "zTl": zTl_p, "aTl": aTl_p,
            "ident": ident,
        })
    return in_maps, (psim, pos, eqmask, ai)


def finish(results, host_ctx):
    """Host-side epilogue: assemble the two losses."""
    psim, pos, eqmask, ai = host_ctx
    terms1 = np.empty(2 * B, dtype=np.float64)
    terms2 = np.empty(P, dtype=np.float64)
    lanes = np.arange(128)
    bounds = np.asarray(UNIT_BOUNDS)
    for c in range(NCORES):
        r = results[c]
        sraw = r["sraw"].astype(np.float64).sum(axis=2)  # [128, 12, 6] -> [128, 12]
        corrA = r["corrA"].astype(np.float64)
        corrD = (np.asarray(r["corrD"], np.int16)
                 .view(ml_dtypes.bfloat16).astype(np.float64))

        for rt in range(RT_SIMCLR):
            if rt < RT_SIMCLR // 2:
                row0 = c * SR + rt * 128                 # z1 rows
            else:
                row0 = B + c * SR + (rt - RT_SIMCLR // 2) * 128
            u = unit_of_col(row0)                        # whole tile in one unit
            corr = corrA[:, rt] if eng_of(rt, u) == "A" else corrD[:, rt]
            s = sraw[:, rt] - corr
            rows = row0 + lanes
            pair = rows % B                              # psim index
            terms1[rows] = np.log(s) - psim[pair]

        for rt in range(RT_SIMCLR, RT):
            p0_ = c * PR + (rt - RT_SIMCLR) * 128
            pg = p0_ + lanes
            ua = np.searchsorted(bounds, ai[pg], side="right") - 1
            isA = np.array([eng_of(rt, int(u)) == "A" for u in ua])
            corr = np.where(isA, corrA[:, rt], corrD[:, rt])
            tot = sraw[:, rt] - corr + eqmask[pg] * np.exp(pos[pg])
            terms2[pg] = np.log(tot) - pos[pg]

    l1 = terms1.mean()
    l2 = terms2.mean()
    return np.array([l1, l2], dtype=np.float32)


def get_nc():
    if "nc" not in _CACHE:
        _CACHE["nc"] = _build_nc()
    return _CACHE["nc"]


def kernel(z1, z2, embeddings, anchor_idx, neighbor_idx):
    in_maps, host_ctx = prepare(z1, z2, embeddings, anchor_idx, neighbor_idx)
    nc = get_nc()
    res = run_bass_kernel_spmd(nc, in_maps, list(range(NCORES)))
    return finish(res.results, host_ctx)


# revision 16
# speedup vs baseline: 1.4918x; 1.0063x over previous
"""Trainium2 Bass kernel for nn_ContrastiveLoss (SimCLR + spatial contrastive loss).

Strategy (8-core data parallel, row-oriented):
  - Host: L2-normalize z1/z2/embeddings (fp32), quantize to fp8e4 (e4m3),
    build transposed [128, 2, cols] operand tables, gather anchor rows,
    compute fp64 positive-pair dots.
  - Device (per core): fp8e4 DoubleRow matmuls (full 256-deep contraction in
    one PE pass at 0.5 cyc/row) of its 1024 simclr rows and 512 spatial rows
    against the full 8192-column tables. The exp+rowsum of each [128, W] PSUM
    tile is split across two engines:
      * ACT: fused exp(x/T) with fp32 accum_out (in-place dead write to PSUM)
      * DVE: Schraudolph bit-trick exp — tensor_scalar computes
        round(A*x + B) into int16 (these ARE the bf16 bits of exp(x/T)),
        then a second 4x-mode tensor_scalar over the bf16 bitcast view
        accumulates the row sums.
    A per-row-tile Gram matmul (same fp8 operands, same DoubleRow mode)
    reproduces the self-similarity diagonal bitwise; it is pushed through
    BOTH engines' exp ops so the host can subtract exactly the value that
    entered each rowsum (engine chosen per row by the static unit map).
  - Host: sum_exp = S_raw - corr(engine-matched), log, subtract fp64
    positives, mean-reduce -> [2] losses.

Self-contained: hardcodes shapes from the problem spec.
"""
import sys

for _p in ("/opt/trn_rl_repo", "/root/.axon_site/_ro/trn_rl_repo"):
    if _p not in sys.path:
        sys.path.insert(0, _p)

import numpy as np
import ml_dtypes

import concourse.tile as tile
from concourse import bacc, mybir
from concourse.bass_utils import run_bass_kernel_spmd

TEMPERATURE = 0.07
B = 4096     # simclr batch
D = 256      # projection dim
N = 8192     # num cells (spatial table rows, also 2B simclr table rows)
P = 4096     # num spatial pairs
NCORES = 8
SR = B // NCORES          # 512 simclr pair-rows per core (=> 1024 sim rows)
PR = P // NCORES          # 512 spatial rows per core
RT_SIMCLR = (2 * SR) // 128   # 8 row-tiles
RT = RT_SIMCLR + PR // 128    # 12 row-tiles total

F32 = mybir.dt.float32
BF16 = mybir.dt.bfloat16
I16 = mybir.dt.int16
FP8E4 = mybir.dt.float8e4

INV_T = float(np.float32(1.0) / np.float32(TEMPERATURE))
# Schraudolph constants: bits16 = round(A16*x + B16) are the bf16 bits of
# ~exp(x/T).  badj calibrated so the weighted mean of the sum ratio is 1.
A16 = float(np.float32(128.0 * np.log2(np.e) / np.float64(np.float32(TEMPERATURE))))
B16 = float(np.float32(127.0 * 128.0 - 10.14))

# --- static schedule configuration -----------------------------------------
PSUM_SIZES = (1536, 1536, 1024)   # psum rotation tile sizes (512-multiples)
GRAM_AT_END = False
GRAM_TILE = -1         # gram outputs fill the last-used psum tiles first
GRAM_REPEAT = 1        # extra idempotent gram passes to warm the PE p-state
ALT_PENALTY = 0.0      # ns penalty for repeating the previous stream engine
SPLIT_C0 = False       # DMA the first table chunk in 512-col pieces
POOL_REDUCE = False    # run SBUF-side reduces on GpSimd instead of DVE
BITS_BUFS = 2


def _mk_units():
    cyc = sum(PSUM_SIZES)
    assert N % cyc == 0
    units = []
    col = 0
    for _ in range(N // cyc):
        for s in PSUM_SIZES:
            units.append((col, s))
            col += s
    return units


UNITS = _mk_units()
NU = len(UNITS)
UNIT_BOUNDS = [u[0] for u in UNITS] + [N]
NBANDS = N // sum(PSUM_SIZES)
NTILES = len(PSUM_SIZES)


def _mk_engine_map():
    """Greedy global load balance across ACT / DVE in program order, with an
    optional bias toward alternating engines between consecutive units."""
    def act_cost(w):
        return 0.8333 * w + 330.0

    def dve_cost(w):
        return 1.0417 * w + 0.26 * w + 335.0

    eng = [[None] * NU for _ in range(RT)]
    ta = td = 0.0
    prev = None
    for band in range(NBANDS):
        for rt in range(RT):
            for j in range(NTILES):
                u = band * NTILES + j
                w = UNITS[u][1]
                ca = ta + act_cost(w) + (ALT_PENALTY if prev == "A" else 0.0)
                cd = td + dve_cost(w) + (ALT_PENALTY if prev == "D" else 0.0)
                if ca <= cd:
                    eng[rt][u] = "A"
                    ta += act_cost(w)
                    prev = "A"
                else:
                    eng[rt][u] = "D"
                    td += dve_cost(w)
                    prev = "D"
    return eng


ENG = _mk_engine_map()


def eng_of(rt, u):
    return ENG[rt][u]


def unit_of_col(col):
    return int(np.searchsorted(UNIT_BOUNDS, col, side="right") - 1)


_CACHE = {}


def _build_nc():
    nc = bacc.Bacc("TRN2", target_bir_lowering=False)

    zT = nc.dram_tensor("zT", [128, 2, N], FP8E4, kind="ExternalInput")
    eT = nc.dram_tensor("eT", [128, 2, N], FP8E4, kind="ExternalInput")
    lT = nc.dram_tensor("lT", [128, 2, 2 * SR + PR], FP8E4, kind="ExternalInput")
    ident = nc.dram_tensor("ident", [128, 128], F32, kind="ExternalInput")

    sraw_o = nc.dram_tensor("sraw", [128, RT, NU], F32, kind="ExternalOutput")
    corrA_o = nc.dram_tensor("corrA", [128, RT], F32, kind="ExternalOutput")
    corrD_o = nc.dram_tensor("corrD", [128, RT], I16, kind="ExternalOutput")

    NCH = 4          # table DMA chunks of 2048 columns
    DR = mybir.MatmulPerfMode.DoubleRow
    WMAX = max(PSUM_SIZES)

    with tile.TileContext(nc) as tc:
        with (
            tc.tile_pool(name="tabs", bufs=1) as tabs,
            tc.tile_pool(name="psum", bufs=1, space="PSUM") as psum,
            tc.tile_pool(name="small", bufs=1) as small,
            tc.tile_pool(name="bits", bufs=BITS_BUFS) as bitsp,
        ):
            lT_t = tabs.tile([128, 2, 2 * SR + PR], FP8E4)
            ident_t = small.tile([128, 128], F32)
            zc = [tabs.tile([128, 2, 2048], FP8E4, name=f"zc{j}")
                  for j in range(NCH)]
            ec = [tabs.tile([128, 2, 2048], FP8E4, name=f"ec{j}")
                  for j in range(NCH)]
            zc0q = ([tabs.tile([128, 2, 512], FP8E4, name=f"zc0q{i}")
                     for i in range(4)] if SPLIT_C0 else None)
            # Load order = consumption order: lhsT slices first (grams +
            # every unit), then the first simclr chunk the first units read.
            nc.sync.dma_start(lT_t[:], lT[:])
            if SPLIT_C0:
                for i in range(4):
                    nc.sync.dma_start(zc0q[i][:],
                                      zT[:, :, i * 512:(i + 1) * 512])
            else:
                nc.sync.dma_start(zc[0][:], zT[:, :, 0:2048])
            nc.sync.dma_start(ident_t[:], ident[:])
            nc.sync.dma_start(ec[0][:], eT[:, :, 0:2048])
            for j in range(1, NCH):
                nc.sync.dma_start(zc[j][:], zT[:, :, j * 2048:(j + 1) * 2048])
                nc.sync.dma_start(ec[j][:], eT[:, :, j * 2048:(j + 1) * 2048])

            def rhs_of(rt, col):
                """[col, col+512) slice of the right table."""
                if rt >= RT_SIMCLR:
                    return ec[col // 2048][:, :, col % 2048:col % 2048 + 512]
                if SPLIT_C0 and col < 2048:
                    return zc0q[col // 512][:]
                return zc[col // 2048][:, :, col % 2048:col % 2048 + 512]

            p_tiles = [psum.tile([128, s], F32, name=f"p{i}")
                       for i, s in enumerate(PSUM_SIZES)]

            part = small.tile([128, RT, NU], F32)
            gd = small.tile([128, RT, 128], F32)
            gdv = small.tile([128, RT], F32)
            corrA_t = small.tile([128, RT], F32)
            corrD_t = small.tile([128, RT], I16)
            sraw_t = small.tile([128, RT], F32)

            def lhsT(rt):
                return lT_t[:, :, rt * 128:(rt + 1) * 128]

            def gram_phase():
                # Gram diagonals == main matmuls' self-similarity elements
                # bitwise (same operands, same DoubleRow mode).
                done = 0
                tile_order = (list(range(NTILES))[::-1] if GRAM_TILE < 0
                              else [(GRAM_TILE + k) % NTILES
                                    for k in range(NTILES)])
                for ti in tile_order:
                    pt = p_tiles[ti]
                    cap = PSUM_SIZES[ti] // 128
                    take = min(cap, RT - done)
                    for rep in range(GRAM_REPEAT):
                        for k in range(take):
                            rt = done + k
                            nc.tensor.matmul(pt[:, k * 128:(k + 1) * 128],
                                             lhsT(rt), lhsT(rt),
                                             start=True, stop=True,
                                             perf_mode=DR)
                    for k in range(take):
                        rt = done + k
                        nc.vector.tensor_tensor(
                            gd[:, rt, :], pt[:, k * 128:(k + 1) * 128],
                            ident_t[:], mybir.AluOpType.mult,
                        )
                    done += take
                    if done >= RT:
                        break
                red = nc.gpsimd if POOL_REDUCE else nc.vector
                red.tensor_reduce(
                    gdv[:], gd[:], axis=mybir.AxisListType.X,
                    op=mybir.AluOpType.add,
                )
                # Exp the diagonals through BOTH engine paths; host selects.
                nc.scalar.activation(
                    corrA_t[:], gdv[:], mybir.ActivationFunctionType.Exp,
                    scale=INV_T,
                )
                nc.vector.tensor_scalar(
                    corrD_t[:], gdv[:], A16, B16,
                    mybir.AluOpType.mult, mybir.AluOpType.add,
                )
                nc.sync.dma_start(corrA_o[:], corrA_t[:])
                nc.sync.dma_start(corrD_o[:], corrD_t[:])

            if not GRAM_AT_END:
                gram_phase()

            # --- Main units: rotation over psum tiles keeps PE ahead of the
            # two exp engines.
            for band in range(NBANDS):
                for rt in range(RT):
                    for j in range(NTILES):
                        u = band * NTILES + j
                        c0, W = UNITS[u]
                        pt = p_tiles[j]
                        for off in range(0, W, 512):
                            nc.tensor.matmul(
                                pt[:, off:off + 512], lhsT(rt),
                                rhs_of(rt, c0 + off),
                                start=True, stop=True, perf_mode=DR,
                            )
                        slot = part[:, rt, u:u + 1]
                        if eng_of(rt, u) == "A":
                            nc.scalar.activation(
                                pt[:, :W], pt[:, :W],
                                mybir.ActivationFunctionType.Exp,
                                scale=INV_T, accum_out=slot,
                            )
                        else:
                            bt = bitsp.tile([128, WMAX], I16, tag="bits")
                            nc.vector.tensor_scalar(
                                bt[:, :W], pt[:, :W], A16, B16,
                                mybir.AluOpType.mult, mybir.AluOpType.add,
                            )
                            bb = bt[:, :W].bitcast(BF16)
                            nc.vector.tensor_scalar(
                                bb, bb, 1.0, 0.0,
                                mybir.AluOpType.mult, mybir.AluOpType.add,
                                accum_out=slot,
                            )

            if GRAM_AT_END:
                gram_phase()

            # Ship the per-unit partial sums; host does the final 6-way add.
            nc.sync.dma_start(sraw_o[:], part[:])

    nc.finalize()
    return nc


def _l2norm(x):
    x = np.asarray(x, dtype=np.float32)
    n = np.maximum(np.linalg.norm(x, axis=1, keepdims=True), 1e-12)
    return (x / n).astype(np.float32)


def _pack_T8(xq):
    """[R, D=256] fp8 -> transposed operand table [128, 2, R] (same bytes)."""
    xT = np.ascontiguousarray(xq.T)                      # [256, R]
    return np.ascontiguousarray(
        xT.reshape(2, 128, xT.shape[1]).transpose(1, 0, 2)
    )


def prepare(z1, z2, embeddings, anchor_idx, neighbor_idx):
    """Host-side prep: returns (in_maps, host_ctx)."""
    z1n = _l2norm(z1)
    z2n = _l2norm(z2)
    en = _l2norm(embeddings)
    ai = np.asarray(anchor_idx).astype(np.int64)
    ni = np.asarray(neighbor_idx).astype(np.int64)

    zcat = np.concatenate([z1n, z2n], axis=0)            # [2B, D] fp32
    zq = zcat.astype(ml_dtypes.float8_e4m3)              # quantize once
    eq8 = en.astype(ml_dtypes.float8_e4m3)

    zT_p = _pack_T8(zq)                                  # [128, 2, 8192]
    eT_p = _pack_T8(eq8)                                 # [128, 2, 8192]
    a_rows8 = eq8[ai]                                    # [P, D] fp8 (same bytes)
    aT_p = _pack_T8(a_rows8)                             # [128, 2, 4096]

    # fp64 positive-pair logits (match reference semantics: full precision)
    psim = (np.sum(z1n.astype(np.float64) * z2n.astype(np.float64), axis=1)
            / np.float64(np.float32(TEMPERATURE)))       # [B]
    pos = (np.sum(en[ai].astype(np.float64) * en[ni].astype(np.float64), axis=1)
           / np.float64(np.float32(TEMPERATURE)))        # [P]
    eqmask = (ai == ni).astype(np.float64)               # [P]

    ident = np.eye(128, dtype=np.float32)
    in_maps = []
    for c in range(NCORES):
        zTl_p = np.ascontiguousarray(np.concatenate(
            [zT_p[:, :, c * SR:(c + 1) * SR],
             zT_p[:, :, B + c * SR:B + (c + 1) * SR]], axis=2))
        aTl_p = np.ascontiguousarray(aT_p[:, :, c * PR:(c + 1) * PR])
        in_maps.append({
            "zT": zT_p, "eT": eT_p,
            "lT": np.ascontiguousarray(
                np.concatenate([zTl_p, aTl_p], axis=2)),
            "ident": ident,
        })
    return in_maps, (psim, pos, eqmask, ai)


def finish(results, host_ctx):
    """Host-side epilogue: assemble the two losses."""
    psim, pos, eqmask, ai = host_ctx
    terms1 = np.empty(2 * B, dtype=np.float64)
    terms2 = np.empty(P, dtype=np.float64)
    lanes = np.arange(128)
    bounds = np.asarray(UNIT_BOUNDS)
    for c in range(NCORES):
        r = results[c]
        sraw = r["sraw"].astype(np.float64).sum(axis=2)  # [128, 12, 6] -> [128, 12]
        corrA = r["corrA"].astype(np.float64)
        corrD = (np.asarray(r["corrD"], np.int16)
                 .view(ml_dtypes.bfloat16).astype(np.float64))

        for rt in range(RT_SIMCLR):
            if rt < RT_SIMCLR // 2:
                row0 = c * SR + rt * 128                 # z1 rows
            else:
                row0 = B + c * SR + (rt - RT_SIMCLR // 2) * 128
            u = unit_of_col(row0)                        # whole tile in one unit
            corr = corrA[:, rt] if eng_of(rt, u) == "A" else corrD[:, rt]
            s = sraw[:, rt] - corr
            rows = row0 + lanes
            pair = rows % B                              # psim index
            terms1[rows] = np.log(s) - psim[pair]

        for rt in range(RT_SIMCLR, RT):
            p0_ = c * PR + (rt - RT_SIMCLR) * 128
            pg = p0_ + lanes
            ua = np.searchsorted(bounds, ai[pg], side="right") - 1
            isA = np.array([eng_of(rt, int(u)) == "A" for u in ua])
            corr = np.where(isA, corrA[:, rt], corrD[:, rt])
            tot = sraw[:, rt] - corr + eqmask[pg] * np.exp(pos[pg])
            terms2[pg] = np.log(tot) - pos[pg]

    l1 = terms1.mean()
    l2 = terms2.mean()
    return np.array([l1, l2], dtype=np.float32)


def get_nc():
    if "nc" not in _CACHE:
        _CACHE["nc"] = _build_nc()
    return _CACHE["nc"]


def kernel(z1, z2, embeddings, anchor_idx, neighbor_idx):
    in_maps, host_ctx = prepare(z1, z2, embeddings, anchor_idx, neighbor_idx)
    nc = get_nc()
    res = run_bass_kernel_spmd(nc, in_maps, list(range(NCORES)))
    return finish(res.results, host_ctx)
